# revision 14
# baseline (speedup 1.0000x reference)
"""Trainium2 Bass kernel for nn_MultiHeadAttention (B=2, S=2048, D=512, H=8).

Sharding: 8 cores = 2 batches x 4 head-pairs. Each core computes, for its
batch b and its 2 heads, the full attention and a partial output projection
(row-parallel W_O); the host sums the 4 partial outputs per batch.

Device-side dataflow per core (all matmuls bf16, fp32 accumulation):
  - inputs arrive as x^T [D, S] (feature-major), weights pre-sliced/transposed
  - Q^T, K^T [128, S] feature-major (2 heads stacked on partitions)
  - V token-major via PE transpose, augmented with a ones column per head so
    the PV matmul also produces the softmax denominator for free
  - scores^T [k, q] per k-tile, causal tiles only; exp on ScalarE with the
    column-padding additive mask folded into the activation bias
  - normalization deferred to after PV: the 4 q-chunk denominators are packed
    onto partitions {0,32,64,96}, reciprocal'd in a 32x32-transposed layout
    (avoids 1-lane DVE reciprocals), row-padding mask folded in
  - out = attn^T.T @ W_O^T + b_O/4, token-major, partial over heads

Constants (iotas, triangular mask, identity) are host-provided inputs so the
GpSimd engine only does DMA descriptor generation and a few broadcasts (its
library swaps between op kinds cost ~6us each).
"""

import sys

for _p in ("/opt/trn_rl_repo",):
    if _p not in sys.path:
        sys.path.insert(0, _p)

import numpy as np

B, S, D, H, DK = 2, 2048, 512, 8, 64
NCORES = 8
HPC = 2            # heads per core
DKH = DK * HPC     # 128: stacked head dim per core
KT = 128           # k-tile (partition tile of keys)
P = 128

_programs = {}


def _consts(s):
    NKT = s // KT
    QCH = min(512, s)
    MW = s // P        # mask columns when loaded partition-major
    RVW = QCH // 32    # columns of the packed rowvalid/denominator layout
    return NKT, QCH, MW, RVW


def make_consts(s):
    """Host-side constant tensors (iotas / masks / identity)."""
    NKT, QCH, MW, RVW = _consts(s)
    p = np.arange(P)
    # cf32: [128, RVW+NKT+1] f32: iota for rv4T compare, iota over k, ones col
    iota_rv = (QCH * (p[:, None] // 32) + (p[:, None] % 32)
               + 32 * np.arange(RVW)[None, :]).astype(np.float32)
    iota_k = (p[:, None] + P * np.arange(NKT)[None, :]).astype(np.float32)
    ones_col = np.ones((P, 1), np.float32)
    cf32 = np.concatenate([iota_rv, iota_k, ones_col], axis=1)
    # cbf: [128, 256] bf16: upper-triangular keep-mask, identity
    import ml_dtypes
    utri = np.triu(np.ones((P, P))).astype(ml_dtypes.bfloat16)
    ident = np.eye(P).astype(ml_dtypes.bfloat16)
    cbf = np.concatenate([utri, ident], axis=1)
    return cf32, cbf


def build_program(s=S):
    import concourse.mybir as mybir
    import concourse.tile as tile
    from concourse import bacc
    from concourse.bass import ts

    dt = mybir.dt
    f32, bf16, i32 = dt.float32, dt.bfloat16, dt.int32
    AF = mybir.ActivationFunctionType
    OP = mybir.AluOpType

    NKT, QCH, MW, RVW = _consts(s)
    NQC = s // QCH              # (must be <= 4 for the denominator packing)
    SCH = min(1024, s)          # q-chunk for score tiles / exp
    PCH = min(512, s)           # q-chunk for projections
    NDC = D // P                # 4 chunks of the contraction dim
    VSTR = 2 * (DK + 1)         # 130: per-k-tile stride in V_aug

    nc = bacc.Bacc("TRN2", target_bir_lowering=False, debug=False,
                   num_devices=NCORES)

    xq_d = nc.dram_tensor("xq", [D, s], f32, kind="ExternalInput")
    xk_d = nc.dram_tensor("xk", [D, s], f32, kind="ExternalInput")
    xv_d = nc.dram_tensor("xv", [D, s], f32, kind="ExternalInput")
    wq_d = nc.dram_tensor("wq", [D, DKH], f32, kind="ExternalInput")
    wk_d = nc.dram_tensor("wk", [D, DKH], f32, kind="ExternalInput")
    wv_d = nc.dram_tensor("wv", [D, DKH], f32, kind="ExternalInput")
    wo_d = nc.dram_tensor("wo", [DKH, D], f32, kind="ExternalInput")
    bqkv_d = nc.dram_tensor("bqkv", [DKH, 3], f32, kind="ExternalInput")
    bo_d = nc.dram_tensor("bo", [D], f32, kind="ExternalInput")
    # masks packed partition-major: [128, 2*MW] i32 (row | col)
    masks_d = nc.dram_tensor("masks", [P, 2 * MW], i32, kind="ExternalInput")
    cf32_d = nc.dram_tensor("cf32", [P, RVW + NKT + 1], f32,
                           kind="ExternalInput")
    cbf_d = nc.dram_tensor("cbf", [P, 2 * P], bf16, kind="ExternalInput")
    out_d = nc.dram_tensor("out", [s, D], f32, kind="ExternalOutput")

    with tile.TileContext(nc) as tc:
        with (
            tc.tile_pool(name="persist", bufs=1) as pe,
            tc.tile_pool(name="pt", bufs=4) as ptp,
            tc.tile_pool(name="sm", bufs=2) as smp,
            tc.tile_pool(name="outp", bufs=3) as outp,
            tc.tile_pool(name="sc", bufs=2, space="PSUM") as scp,
            tc.tile_pool(name="at", bufs=4, space="PSUM") as atp,
        ):
            # ------- small constants via HWDGE (sync) ----------------------
            bqkv = pe.tile([P, 3], f32, tag="bqkv")
            nc.sync.dma_start(bqkv[:], bqkv_d[:])
            masks = pe.tile([P, 2 * MW], i32, tag="masks")
            nc.sync.dma_start(masks[:], masks_d[:])
            cf32 = pe.tile([P, RVW + NKT + 1], f32, tag="cf32")
            nc.sync.dma_start(cf32[:], cf32_d[:])
            cbf = pe.tile([P, 2 * P], bf16, tag="cbf")
            nc.sync.dma_start(cbf[:], cbf_d[:])
            wo_f = pe.tile([P, D], f32, tag="wo_f")
            nc.sync.dma_start(wo_f[:], wo_d[:])
            bo_sb = pe.tile([1, D], f32, tag="bo")
            nc.sync.dma_start(bo_sb[0:1, :], bo_d[None, :])
            iota_rv = cf32[:, 0:RVW]
            iota_k = cf32[:, RVW:RVW + NKT]
            ones_col = cf32[:, RVW + NKT:RVW + NKT + 1]
            utri = cbf[:, 0:P]
            ident = cbf[:, P:2 * P]

            # ------- x^T + weight loads (SWDGE, casts f32->bf16) -----------
            xbf = {}
            wsb = {}
            for nm, xd, wd in (("xq", xq_d, wq_d), ("xk", xk_d, wk_d),
                               ("xv", xv_d, wv_d)):
                w = pe.tile([P, NDC, DKH], bf16, tag="w" + nm[1])
                nc.gpsimd.dma_start(w[:], wd[:].rearrange("(c p) m -> p c m", p=P))
                wsb["w" + nm[1]] = w
                xbf[nm] = []
                for c in range(NDC):
                    t = pe.tile([P, s], bf16, tag=f"{nm}{c}")
                    nc.gpsimd.dma_start(t[:], xd[ts(c, P), :])
                    xbf[nm].append(t)

            # ------- mask lengths + compare masks (wide ops only) ----------
            masks_f = pe.tile([P, 2 * MW], f32, tag="masks_f")
            nc.vector.tensor_copy(out=masks_f[:], in_=masks[:])
            msum = pe.tile([P, 2], f32, tag="msum")
            nc.vector.tensor_reduce(msum[:, 0:1], masks_f[:, 0:MW],
                                    axis=mybir.AxisListType.X, op=OP.add)
            nc.vector.tensor_reduce(msum[:, 1:2], masks_f[:, MW:2 * MW],
                                    axis=mybir.AxisListType.X, op=OP.add)
            # sum over partitions + broadcast back, via tiny f32 matmuls
            lens_ps = scp.tile([P, SCH], f32, tag="sc", name="lens_ps")
            nc.tensor.matmul(lens_ps[0:1, 0:2], ones_col, msum[:, 0:2],
                             start=True, stop=True)
            lens = pe.tile([1, 2], f32, tag="lens")
            nc.vector.tensor_copy(out=lens[0:1, :], in_=lens_ps[0:1, 0:2])
            ones_row = pe.tile([1, P], f32, tag="ones_row")
            nc.vector.memset(ones_row[0:1, :], 1.0)
            lbc_ps = scp.tile([P, SCH], f32, tag="sc", name="lbc_ps")
            nc.tensor.matmul(lbc_ps[:, 0:2], ones_row[0:1, :], lens[0:1, :],
                             start=True, stop=True)
            lens_bc = pe.tile([P, 2], f32, tag="lens_bc")
            nc.vector.tensor_copy(out=lens_bc[:], in_=lbc_ps[:, 0:2])

            # rv4T: rowvalid in the 32x32-transposed packed layout [128, MW]
            rv4T = pe.tile([P, RVW], f32, tag="rv4T")
            nc.vector.tensor_scalar(rv4T[:, :], iota_rv, lens_bc[:, 0:1],
                                    None, OP.is_lt)
            # colmask: 0 valid / -1e30 masked, [128, NKT]
            colmask = pe.tile([P, NKT], f32, tag="colmask")
            nc.vector.tensor_scalar(colmask[:, :], iota_k, lens_bc[:, 1:2],
                                    None, OP.is_lt)
            nc.vector.tensor_scalar(colmask[:, :], colmask[:, :],
                                    1.0, 1e30, OP.subtract, OP.mult)

            den4 = pe.tile([P, QCH], f32, tag="den4")
            nc.vector.memset(den4[:], 1.0)
            den4T = pe.tile([P, QCH], f32, tag="den4T")
            den4T_s = den4T[:].rearrange("p (j b) -> p j b", b=32)[:, :, 0]

            # ---------------- projections ----------------
            QT = pe.tile([P, s], bf16, tag="QT")
            KTt = pe.tile([P, s], bf16, tag="KTt")
            VT = pe.tile([P, s], bf16, tag="VT")
            for dst, wnm, bcol, xnm in (
                (QT, "wq", 0, "xq"),
                (KTt, "wk", 1, "xk"),
                (VT, "wv", 2, "xv"),
            ):
                for j in range(s // PCH):
                    ps = scp.tile([P, SCH], f32, tag="sc", name=f"pj_{wnm}_{j}")
                    for c in range(NDC):
                        nc.tensor.matmul(
                            ps[:, 0:PCH], wsb[wnm][:, c, :],
                            xbf[xnm][c][:, ts(j, PCH)],
                            start=(c == 0), stop=(c == NDC - 1))
                    nc.vector.tensor_scalar(dst[:, ts(j, PCH)], ps[:, 0:PCH],
                                            bqkv[:, bcol:bcol + 1], None,
                                            OP.add)

            # V_aug token-major: per k-tile m, cols [Vh0(64)|1|Vh1(64)|1]
            V_aug = pe.tile([P, NKT * VSTR], bf16, tag="vaug")
            nc.vector.memset(V_aug[:], 1.0)
            for m in range(NKT):
                o = m * VSTR
                tp = scp.tile([P, P], bf16, tag="sc", name=f"tp_{m}")
                nc.tensor.transpose(tp[:, 0:P], VT[:, ts(m, P)], ident)
                nc.vector.tensor_copy(out=V_aug[:, o:o + DK], in_=tp[:, 0:DK])
                nc.vector.tensor_copy(out=V_aug[:, o + DK + 1:o + 2 * DK + 1],
                                      in_=tp[:, DK:2 * DK])

            wo_bf = pe.tile([P, D], bf16, tag="wo_bf")
            nc.vector.tensor_copy(out=wo_bf[:], in_=wo_f[:])
            bo_ps = scp.tile([P, max(SCH, D)], f32, tag="sc", name="bo_ps")
            nc.tensor.matmul(bo_ps[:, 0:D], ones_row[0:1, :], bo_sb[0:1, :],
                             start=True, stop=True)
            bo_bc = pe.tile([P, D], f32, tag="bo_bc")
            nc.vector.tensor_copy(out=bo_bc[:], in_=bo_ps[:, 0:D])

            # ---------------- attention (per head) ----------------
            attnT = pe.tile([P, s], bf16, tag="attnT")

            def emit_out_tile(t):
                ps = scp.tile([P, max(SCH, D)], f32, tag="sc",
                              name=f"out_ps_{t}")
                nc.tensor.matmul(ps[:, 0:D], attnT[:, ts(t, P)], wo_bf[:, :],
                                 start=True, stop=True)
                os_ = outp.tile([P, D], f32, tag="os", name=f"os_{t}")
                nc.vector.tensor_tensor(os_[:, :], ps[:, 0:D], bo_bc[:, :],
                                        OP.add)
                nc.sync.dma_start(out_d[ts(t, P), :], os_[:, :])

            def emit_qk_exp(h, m, pts):
                hb = h * DK
                q0 = m * KT
                width = s - q0
                PT_m = ptp.tile([P, s], bf16, tag="pt", name=f"pt_{h}_{m}")
                pts[m] = PT_m
                for off in range(0, width, SCH):
                    n = min(SCH, width - off)
                    ps = scp.tile([P, SCH], f32, tag="sc",
                                  name=f"sc_{h}_{m}_{off}")
                    for o2 in range(0, n, 512):
                        n2 = min(512, n - o2)
                        nc.tensor.matmul(
                            ps[:, o2:o2 + n2], KTt[hb:hb + DK, ts(m, KT)],
                            QT[hb:hb + DK, q0 + off + o2:q0 + off + o2 + n2],
                            start=True, stop=True, skip_group_check=True)
                    nc.scalar.activation(
                        PT_m[:, off:off + n], ps[:, 0:n], AF.Exp,
                        bias=colmask[:, m:m + 1], scale=1.0)

            for h in range(HPC):
                hb = h * DK
                aps = [atp.tile([P, QCH], f32, tag="at", name=f"aps_h{h}_{i}")
                       for i in range(NQC)]
                pts = {}
                emit_qk_exp(h, 0, pts)
                for m in range(NKT):
                    q0 = m * KT
                    if m + 1 < NKT:
                        emit_qk_exp(h, m + 1, pts)
                    PT_m = pts.pop(m)
                    # causal mask on the diagonal block
                    nc.vector.tensor_tensor(PT_m[:, 0:P], PT_m[:, 0:P],
                                            utri, OP.mult)
                    # PV accumulate (+ denominator at row 64 via ones column)
                    for qc in range(NQC):
                        aq0 = qc * QCH
                        if q0 >= aq0 + QCH:
                            continue
                        lo = max(aq0, q0)
                        psoff = lo - aq0
                        n = aq0 + QCH - lo
                        m_last = (aq0 + QCH) // KT - 1
                        nc.tensor.matmul(
                            aps[qc][0:DK + 1, psoff:psoff + n],
                            V_aug[:, m * VSTR + h * (DK + 1):
                                  m * VSTR + (h + 1) * (DK + 1)],
                            PT_m[:, lo - q0:lo - q0 + n],
                            start=(m == 0), stop=(m == m_last),
                            skip_group_check=True)
                        if m == m_last:
                            nc.vector.tensor_copy(
                                out=den4[32 * qc:32 * qc + 1, :],
                                in_=aps[qc][DK:DK + 1, :])
                    # after the last contributor of a half: normalize that
                    # half (wide reciprocal in 32x32-transposed layout) and,
                    # on the last head, emit its output tiles
                    for half in range(max(1, NQC // 2)):
                        qlo = half * 2
                        qhi = min(NQC, qlo + 2)
                        if m != (qhi * QCH) // KT - 1:
                            continue
                        pb = 32 * qlo
                        pn = 32 * (qhi - qlo)
                        nc.vector.transpose(den4T[pb:pb + pn, :],
                                            den4[pb:pb + pn, :])
                        d_s = den4T[pb:pb + pn, :].rearrange(
                            "p (j b) -> p j b", b=32)[:, :, 0]
                        nc.vector.reciprocal(d_s, d_s)
                        nc.vector.tensor_tensor(d_s, d_s, rv4T[pb:pb + pn, :],
                                                OP.mult)
                        for qc in range(qlo, qhi):
                            rmq = smp.tile([32, QCH], f32, tag="rmq",
                                           name=f"rmq_{h}_{qc}")
                            nc.vector.transpose(
                                rmq[0:32, :], den4T[32 * qc:32 * qc + 32, :])
                            rbc = smp.tile([DK, QCH], f32, tag="rbc",
                                           name=f"rbc_{h}_{qc}")
                            nc.gpsimd.partition_broadcast(rbc[:, :],
                                                          rmq[0:1, :])
                            nc.vector.tensor_tensor(
                                attnT[hb:hb + DK, ts(qc, QCH)],
                                aps[qc][0:DK, :], rbc[0:DK, :], OP.mult)
                            if h == HPC - 1:
                                for t in range(qc * QCH // P,
                                               (qc + 1) * QCH // P):
                                    emit_out_tile(t)

    nc.compile()
    return nc


def _get_program(s=S):
    if s not in _programs:
        _programs[s] = build_program(s)
    return _programs[s]


def pack_masks(row_pad_mask, col_pad_mask, s=S):
    MW = s // P
    r = np.asarray(row_pad_mask, np.int32).reshape(MW, P).T
    c = np.asarray(col_pad_mask, np.int32).reshape(MW, P).T
    return np.ascontiguousarray(np.concatenate([r, c], axis=1))


def make_in_maps(in_Q, in_K, in_V, row_pad_mask, col_pad_mask,
                 W_Q, b_Q, W_K, b_K, W_V, b_V, W_O, b_O):
    """Shard the full inputs into the 8 per-core input maps."""
    f = np.float32
    cf32, cbf = make_consts(S)
    xT = {}
    masks = {}
    for b in range(B):
        xT[b] = tuple(
            np.ascontiguousarray(np.asarray(x[b], dtype=f).T)
            for x in (in_Q, in_K, in_V))
        masks[b] = pack_masks(row_pad_mask[b], col_pad_mask[b])
    per_hp = []
    for hp in range(NCORES // B):
        sl = slice(hp * DKH, (hp + 1) * DKH)
        bqkv = np.stack([np.asarray(b_Q, f)[sl] / 8.0,
                         np.asarray(b_K, f)[sl],
                         np.asarray(b_V, f)[sl]], axis=1)
        per_hp.append(dict(
            wq=np.ascontiguousarray(np.asarray(W_Q, f)[sl, :].T) / 8.0,
            wk=np.ascontiguousarray(np.asarray(W_K, f)[sl, :].T),
            wv=np.ascontiguousarray(np.asarray(W_V, f)[sl, :].T),
            wo=np.ascontiguousarray(np.asarray(W_O, f)[:, sl].T),
            bqkv=np.ascontiguousarray(bqkv),
            bo=np.asarray(b_O, f) / 4.0,
            cf32=cf32, cbf=cbf,
        ))
    in_maps = []
    for c in range(NCORES):
        b, hp = divmod(c, NCORES // B)
        m = dict(per_hp[hp])
        m["xq"], m["xk"], m["xv"] = xT[b]
        m["masks"] = masks[b]
        in_maps.append(m)
    return in_maps


def run(in_maps, trace=False, **trace_kwargs):
    from concourse.bass_utils import run_bass_kernel_spmd
    nc = _get_program()
    return run_bass_kernel_spmd(nc, in_maps, core_ids=list(range(NCORES)),
                                trace=trace, **trace_kwargs)


def kernel(**inputs):
    in_maps = make_in_maps(**inputs)
    res = run(in_maps)
    out = np.zeros((B, S, D), np.float32)
    for c in range(NCORES):
        b = c // (NCORES // B)
        out[b] += res.results[c]["out"]
    return out


# revision 15
# speedup vs baseline: 1.0368x; 1.0368x over previous
"""Trainium2 Bass kernel for nn_MultiHeadAttention (B=2, S=2048, D=512, H=8).

Sharding: 8 cores = 2 batches x 4 head-pairs. Each core computes, for its
batch b and its 2 heads, the full attention and a partial output projection
(row-parallel W_O); the host sums the 4 partial outputs per batch.

Device-side dataflow per core (all matmuls bf16, fp32 accumulation):
  - inputs arrive as x^T [D, S] (feature-major), weights pre-sliced/transposed
  - Q^T, K^T [128, S] feature-major (2 heads stacked on partitions)
  - V token-major via PE transpose, augmented with a ones column per head so
    the PV matmul also produces the softmax denominator for free
  - scores^T [k, q] per k-tile, causal tiles only; exp on ScalarE with the
    column-padding additive mask folded into the activation bias
  - normalization deferred to after PV: the 4 q-chunk denominators are packed
    onto partitions {0,32,64,96}, reciprocal'd in a 32x32-transposed layout
    (avoids 1-lane DVE reciprocals), row-padding mask folded in
  - out = attn^T.T @ W_O^T + b_O/4, token-major, partial over heads

Constants (iotas, triangular mask, identity) are host-provided inputs so the
GpSimd engine only does DMA descriptor generation and a few broadcasts (its
library swaps between op kinds cost ~6us each).
"""

import sys

for _p in ("/opt/trn_rl_repo",):
    if _p not in sys.path:
        sys.path.insert(0, _p)

import numpy as np

B, S, D, H, DK = 2, 2048, 512, 8, 64
NCORES = 8
HPC = 2            # heads per core
DKH = DK * HPC     # 128: stacked head dim per core
KT = 128           # k-tile (partition tile of keys)
P = 128

_programs = {}


def _consts(s):
    NKT = s // KT
    QCH = min(512, s)
    MW = s // P        # mask columns when loaded partition-major
    RVW = QCH // 32    # columns of the packed rowvalid/denominator layout
    return NKT, QCH, MW, RVW


def make_consts(s):
    """Host-side constant tensors (iotas / masks / identity)."""
    NKT, QCH, MW, RVW = _consts(s)
    p = np.arange(P)
    # cf32: [128, RVW+NKT+1] f32: iota for rv4T compare, iota over k, ones col
    iota_rv = (QCH * (p[:, None] // 32) + (p[:, None] % 32)
               + 32 * np.arange(RVW)[None, :]).astype(np.float32)
    iota_k = (p[:, None] + P * np.arange(NKT)[None, :]).astype(np.float32)
    ones_col = np.ones((P, 1), np.float32)
    cf32 = np.concatenate([iota_rv, iota_k, ones_col], axis=1)
    # cbf: [128, 256] bf16: upper-triangular keep-mask, identity
    import ml_dtypes
    utri = np.triu(np.ones((P, P))).astype(ml_dtypes.bfloat16)
    ident = np.eye(P).astype(ml_dtypes.bfloat16)
    cbf = np.concatenate([utri, ident], axis=1)
    return cf32, cbf


def build_program(s=S):
    import concourse.mybir as mybir
    import concourse.tile as tile
    from concourse import bacc
    from concourse.bass import ts

    dt = mybir.dt
    f32, bf16, i32 = dt.float32, dt.bfloat16, dt.int32
    AF = mybir.ActivationFunctionType
    OP = mybir.AluOpType

    NKT, QCH, MW, RVW = _consts(s)
    NQC = s // QCH              # (must be <= 4 for the denominator packing)
    SCH = min(1024, s)          # q-chunk for score tiles / exp
    PCH = min(512, s)           # q-chunk for projections
    NDC = D // P                # 4 chunks of the contraction dim
    VSTR = 2 * (DK + 1)         # 130: per-k-tile stride in V_aug

    nc = bacc.Bacc("TRN2", target_bir_lowering=False, debug=False,
                   num_devices=NCORES)

    xq_d = nc.dram_tensor("xq", [D, s], f32, kind="ExternalInput")
    xk_d = nc.dram_tensor("xk", [D, s], f32, kind="ExternalInput")
    xv_d = nc.dram_tensor("xv", [D, s], f32, kind="ExternalInput")
    wq_d = nc.dram_tensor("wq", [D, DKH], f32, kind="ExternalInput")
    wk_d = nc.dram_tensor("wk", [D, DKH], f32, kind="ExternalInput")
    wv_d = nc.dram_tensor("wv", [D, DKH], f32, kind="ExternalInput")
    wo_d = nc.dram_tensor("wo", [DKH, D], f32, kind="ExternalInput")
    bqkv_d = nc.dram_tensor("bqkv", [DKH, 3], f32, kind="ExternalInput")
    bo_d = nc.dram_tensor("bo", [D], f32, kind="ExternalInput")
    # masks packed partition-major: [128, 2*MW] i32 (row | col)
    masks_d = nc.dram_tensor("masks", [P, 2 * MW], i32, kind="ExternalInput")
    cf32_d = nc.dram_tensor("cf32", [P, RVW + NKT + 1], f32,
                           kind="ExternalInput")
    cbf_d = nc.dram_tensor("cbf", [P, 2 * P], bf16, kind="ExternalInput")
    out_d = nc.dram_tensor("out", [s, D], f32, kind="ExternalOutput")

    with tile.TileContext(nc) as tc:
        with (
            tc.tile_pool(name="persist", bufs=1) as pe,
            tc.tile_pool(name="pt", bufs=4) as ptp,
            tc.tile_pool(name="sm", bufs=2) as smp,
            tc.tile_pool(name="outp", bufs=3) as outp,
            tc.tile_pool(name="sc", bufs=2, space="PSUM") as scp,
            tc.tile_pool(name="at", bufs=4, space="PSUM") as atp,
        ):
            # ------- small constants via HWDGE (sync) ----------------------
            bqkv = pe.tile([P, 3], f32, tag="bqkv")
            nc.sync.dma_start(bqkv[:], bqkv_d[:])
            masks = pe.tile([P, 2 * MW], i32, tag="masks")
            nc.sync.dma_start(masks[:], masks_d[:])
            cf32 = pe.tile([P, RVW + NKT + 1], f32, tag="cf32")
            nc.sync.dma_start(cf32[:], cf32_d[:])
            cbf = pe.tile([P, 2 * P], bf16, tag="cbf")
            nc.sync.dma_start(cbf[:], cbf_d[:])
            wo_f = pe.tile([P, D], f32, tag="wo_f")
            nc.sync.dma_start(wo_f[:], wo_d[:])
            bo_sb = pe.tile([1, D], f32, tag="bo")
            nc.sync.dma_start(bo_sb[0:1, :], bo_d[None, :])
            iota_rv = cf32[:, 0:RVW]
            iota_k = cf32[:, RVW:RVW + NKT]
            ones_col = cf32[:, RVW + NKT:RVW + NKT + 1]
            utri = cbf[:, 0:P]
            ident = cbf[:, P:2 * P]

            # ------- x^T + weight loads (SWDGE, casts f32->bf16) -----------
            xbf = {}
            wsb = {}
            for nm, xd, wd in (("xq", xq_d, wq_d), ("xk", xk_d, wk_d),
                               ("xv", xv_d, wv_d)):
                w = pe.tile([P, NDC, DKH], bf16, tag="w" + nm[1])
                nc.gpsimd.dma_start(w[:], wd[:].rearrange("(c p) m -> p c m", p=P))
                wsb["w" + nm[1]] = w
                xbf[nm] = []
                for c in range(NDC):
                    t = pe.tile([P, s], bf16, tag=f"{nm}{c}")
                    nc.gpsimd.dma_start(t[:], xd[ts(c, P), :])
                    xbf[nm].append(t)

            # ------- mask lengths + compare masks (wide ops only) ----------
            masks_f = pe.tile([P, 2 * MW], f32, tag="masks_f")
            nc.vector.tensor_copy(out=masks_f[:], in_=masks[:])
            msum = pe.tile([P, 2], f32, tag="msum")
            nc.vector.tensor_reduce(msum[:, 0:1], masks_f[:, 0:MW],
                                    axis=mybir.AxisListType.X, op=OP.add)
            nc.vector.tensor_reduce(msum[:, 1:2], masks_f[:, MW:2 * MW],
                                    axis=mybir.AxisListType.X, op=OP.add)
            # sum over partitions + broadcast back, via tiny f32 matmuls
            lens_ps = scp.tile([P, SCH], f32, tag="sc", name="lens_ps")
            nc.tensor.matmul(lens_ps[0:1, 0:2], ones_col, msum[:, 0:2],
                             start=True, stop=True)
            lens = pe.tile([1, 2], f32, tag="lens")
            nc.vector.tensor_copy(out=lens[0:1, :], in_=lens_ps[0:1, 0:2])
            ones_row = pe.tile([1, P], f32, tag="ones_row")
            nc.vector.memset(ones_row[0:1, :], 1.0)
            lbc_ps = scp.tile([P, SCH], f32, tag="sc", name="lbc_ps")
            nc.tensor.matmul(lbc_ps[:, 0:2], ones_row[0:1, :], lens[0:1, :],
                             start=True, stop=True)
            lens_bc = pe.tile([P, 2], f32, tag="lens_bc")
            nc.vector.tensor_copy(out=lens_bc[:], in_=lbc_ps[:, 0:2])

            # rv4T: rowvalid in the 32x32-transposed packed layout [128, MW]
            rv4T = pe.tile([P, RVW], f32, tag="rv4T")
            nc.vector.tensor_scalar(rv4T[:, :], iota_rv, lens_bc[:, 0:1],
                                    None, OP.is_lt)
            # colmask: 0 valid / -1e30 masked, [128, NKT]
            colmask = pe.tile([P, NKT], f32, tag="colmask")
            nc.vector.tensor_scalar(colmask[:, :], iota_k, lens_bc[:, 1:2],
                                    None, OP.is_lt)
            nc.vector.tensor_scalar(colmask[:, :], colmask[:, :],
                                    1.0, 1e30, OP.subtract, OP.mult)

            den4 = pe.tile([P, QCH], f32, tag="den4")
            nc.vector.memset(den4[:], 1.0)
            den4T = pe.tile([P, QCH], f32, tag="den4T")
            den4T_s = den4T[:].rearrange("p (j b) -> p j b", b=32)[:, :, 0]

            # ---------------- projections ----------------
            QT = pe.tile([P, s], bf16, tag="QT")
            KTt = pe.tile([P, s], bf16, tag="KTt")
            VT = pe.tile([P, s], bf16, tag="VT")
            for dst, wnm, bcol, xnm in (
                (QT, "wq", 0, "xq"),
                (KTt, "wk", 1, "xk"),
                (VT, "wv", 2, "xv"),
            ):
                for j in range(s // PCH):
                    ps = scp.tile([P, SCH], f32, tag="sc", name=f"pj_{wnm}_{j}")
                    for c in range(NDC):
                        nc.tensor.matmul(
                            ps[:, 0:PCH], wsb[wnm][:, c, :],
                            xbf[xnm][c][:, ts(j, PCH)],
                            start=(c == 0), stop=(c == NDC - 1))
                    nc.vector.tensor_scalar(dst[:, ts(j, PCH)], ps[:, 0:PCH],
                                            bqkv[:, bcol:bcol + 1], None,
                                            OP.add)

            # V_aug token-major: per k-tile m, cols [Vh0(64)|1|Vh1(64)|1]
            V_aug = pe.tile([P, NKT * VSTR], bf16, tag="vaug")
            nc.vector.memset(V_aug[:], 1.0)
            for m in range(NKT):
                o = m * VSTR
                tp = scp.tile([P, P], bf16, tag="sc", name=f"tp_{m}")
                nc.tensor.transpose(tp[:, 0:P], VT[:, ts(m, P)], ident)
                nc.vector.tensor_copy(out=V_aug[:, o:o + DK], in_=tp[:, 0:DK])
                nc.vector.tensor_copy(out=V_aug[:, o + DK + 1:o + 2 * DK + 1],
                                      in_=tp[:, DK:2 * DK])

            wo_bf = pe.tile([P, D], bf16, tag="wo_bf")
            nc.vector.tensor_copy(out=wo_bf[:], in_=wo_f[:])
            bo_ps = scp.tile([P, max(SCH, D)], f32, tag="sc", name="bo_ps")
            nc.tensor.matmul(bo_ps[:, 0:D], ones_row[0:1, :], bo_sb[0:1, :],
                             start=True, stop=True)
            bo_bc = pe.tile([P, D], f32, tag="bo_bc")
            nc.vector.tensor_copy(out=bo_bc[:], in_=bo_ps[:, 0:D])

            # ---------------- attention (per head) ----------------
            attnT = pe.tile([P, s], bf16, tag="attnT")

            def emit_out_tile(t):
                ps = scp.tile([P, max(SCH, D)], f32, tag="sc",
                              name=f"out_ps_{t}")
                nc.tensor.matmul(ps[:, 0:D], attnT[:, ts(t, P)], wo_bf[:, :],
                                 start=True, stop=True)
                os_ = outp.tile([P, D], f32, tag="os", name=f"os_{t}")
                nc.vector.tensor_tensor(os_[:, :], ps[:, 0:D], bo_bc[:, :],
                                        OP.add)
                nc.sync.dma_start(out_d[ts(t, P), :], os_[:, :])

            # HAM warm-up: a dense block of dependency-chained matmuls
            # (WAW on one psum tile) so the PE sees one fully-busy SHORT
            # window and unthrottles to 2.4 GHz before attention starts.
            wu = scp.tile([P, SCH], f32, tag="sc", name="warmup_ps")
            for i in range(24):
                nc.tensor.matmul(wu[:, 0:P], ident, ident,
                                 start=True, stop=True, skip_group_check=True)

            for h in range(HPC):
                hb = h * DK
                aps = [atp.tile([P, QCH], f32, tag="at", name=f"aps_h{h}_{i}")
                       for i in range(NQC)]
                for m in range(NKT):
                    q0 = m * KT
                    width = s - q0
                    PT_m = ptp.tile([P, s], bf16, tag="pt", name=f"pt_{h}_{m}")
                    for off in range(0, width, SCH):
                        n = min(SCH, width - off)
                        ps = scp.tile([P, SCH], f32, tag="sc",
                                      name=f"sc_{h}_{m}_{off}")
                        for o2 in range(0, n, 512):
                            n2 = min(512, n - o2)
                            nc.tensor.matmul(
                                ps[:, o2:o2 + n2], KTt[hb:hb + DK, ts(m, KT)],
                                QT[hb:hb + DK, q0 + off + o2:q0 + off + o2 + n2],
                                start=True, stop=True, skip_group_check=True)
                        nc.scalar.activation(
                            PT_m[:, off:off + n], ps[:, 0:n], AF.Exp,
                            bias=colmask[:, m:m + 1], scale=1.0)
                    # causal mask on the diagonal block
                    nc.vector.tensor_tensor(PT_m[:, 0:P], PT_m[:, 0:P],
                                            utri, OP.mult)
                    # PV accumulate (+ denominator at row 64 via ones column)
                    for qc in range(NQC):
                        aq0 = qc * QCH
                        if q0 >= aq0 + QCH:
                            continue
                        lo = max(aq0, q0)
                        psoff = lo - aq0
                        n = aq0 + QCH - lo
                        m_last = (aq0 + QCH) // KT - 1
                        nc.tensor.matmul(
                            aps[qc][0:DK + 1, psoff:psoff + n],
                            V_aug[:, m * VSTR + h * (DK + 1):
                                  m * VSTR + (h + 1) * (DK + 1)],
                            PT_m[:, lo - q0:lo - q0 + n],
                            start=(m == 0), stop=(m == m_last),
                            skip_group_check=True)
                        if m == m_last:
                            # denominator -> packed row 32*qc, then normalize
                            # this q-chunk (wide reciprocal in 32x32-transposed
                            # layout); on the last head emit its output tiles
                            nc.vector.tensor_copy(
                                out=den4[32 * qc:32 * qc + 1, :],
                                in_=aps[qc][DK:DK + 1, :])
                            pb = 32 * qc
                            nc.vector.transpose(den4T[pb:pb + 32, :],
                                                den4[pb:pb + 32, :])
                            d_s = den4T[pb:pb + 32, :].rearrange(
                                "p (j b) -> p j b", b=32)[:, :, 0]
                            nc.vector.reciprocal(d_s, d_s)
                            nc.vector.tensor_tensor(d_s, d_s,
                                                    rv4T[pb:pb + 32, :],
                                                    OP.mult)
                            rmq = smp.tile([32, QCH], f32, tag="rmq",
                                           name=f"rmq_{h}_{qc}")
                            nc.vector.transpose(
                                rmq[0:32, :], den4T[pb:pb + 32, :])
                            rbc = smp.tile([DK, QCH], f32, tag="rbc",
                                           name=f"rbc_{h}_{qc}")
                            nc.gpsimd.partition_broadcast(rbc[:, :],
                                                          rmq[0:1, :])
                            nc.vector.tensor_tensor(
                                attnT[hb:hb + DK, ts(qc, QCH)],
                                aps[qc][0:DK, :], rbc[0:DK, :], OP.mult)
                            if h == HPC - 1:
                                for t in range(qc * QCH // P,
                                               (qc + 1) * QCH // P):
                                    emit_out_tile(t)

    nc.compile()
    return nc


def _get_program(s=S):
    if s not in _programs:
        _programs[s] = build_program(s)
    return _programs[s]


def pack_masks(row_pad_mask, col_pad_mask, s=S):
    MW = s // P
    r = np.asarray(row_pad_mask, np.int32).reshape(MW, P).T
    c = np.asarray(col_pad_mask, np.int32).reshape(MW, P).T
    return np.ascontiguousarray(np.concatenate([r, c], axis=1))


def make_in_maps(in_Q, in_K, in_V, row_pad_mask, col_pad_mask,
                 W_Q, b_Q, W_K, b_K, W_V, b_V, W_O, b_O):
    """Shard the full inputs into the 8 per-core input maps."""
    f = np.float32
    cf32, cbf = make_consts(S)
    xT = {}
    masks = {}
    for b in range(B):
        xT[b] = tuple(
            np.ascontiguousarray(np.asarray(x[b], dtype=f).T)
            for x in (in_Q, in_K, in_V))
        masks[b] = pack_masks(row_pad_mask[b], col_pad_mask[b])
    per_hp = []
    for hp in range(NCORES // B):
        sl = slice(hp * DKH, (hp + 1) * DKH)
        bqkv = np.stack([np.asarray(b_Q, f)[sl] / 8.0,
                         np.asarray(b_K, f)[sl],
                         np.asarray(b_V, f)[sl]], axis=1)
        per_hp.append(dict(
            wq=np.ascontiguousarray(np.asarray(W_Q, f)[sl, :].T) / 8.0,
            wk=np.ascontiguousarray(np.asarray(W_K, f)[sl, :].T),
            wv=np.ascontiguousarray(np.asarray(W_V, f)[sl, :].T),
            wo=np.ascontiguousarray(np.asarray(W_O, f)[:, sl].T),
            bqkv=np.ascontiguousarray(bqkv),
            bo=np.asarray(b_O, f) / 4.0,
            cf32=cf32, cbf=cbf,
        ))
    in_maps = []
    for c in range(NCORES):
        b, hp = divmod(c, NCORES // B)
        m = dict(per_hp[hp])
        m["xq"], m["xk"], m["xv"] = xT[b]
        m["masks"] = masks[b]
        in_maps.append(m)
    return in_maps


def run(in_maps, trace=False, **trace_kwargs):
    from concourse.bass_utils import run_bass_kernel_spmd
    nc = _get_program()
    return run_bass_kernel_spmd(nc, in_maps, core_ids=list(range(NCORES)),
                                trace=trace, **trace_kwargs)


def kernel(**inputs):
    in_maps = make_in_maps(**inputs)
    res = run(in_maps)
    out = np.zeros((B, S, D), np.float32)
    for c in range(NCORES):
        b = c // (NCORES // B)
        out[b] += res.results[c]["out"]
    return out


# revision 16
# speedup vs baseline: 1.0980x; 1.0590x over previous
"""Trainium2 Bass kernel for nn_MultiHeadAttention (B=2, S=2048, D=512, H=8).

Sharding: 8 cores = 2 batches x 4 head-pairs. Each core computes, for its
batch b and its 2 heads, the full attention and a partial output projection
(row-parallel W_O); the host sums the 4 partial outputs per batch.

Device-side dataflow per core (all matmuls bf16, fp32 accumulation):
  - inputs arrive as x^T [D, S] (feature-major), weights pre-sliced/transposed
  - Q^T, K^T [128, S] feature-major (2 heads stacked on partitions)
  - V token-major via PE transpose, augmented with a ones column per head so
    the PV matmul also produces the softmax denominator for free
  - scores^T [k, q] per k-tile, causal tiles only; exp on ScalarE with the
    column-padding additive mask folded into the activation bias
  - normalization deferred to after PV: the 4 q-chunk denominators are packed
    onto partitions {0,32,64,96}, reciprocal'd in a 32x32-transposed layout
    (avoids 1-lane DVE reciprocals), row-padding mask folded in
  - out = attn^T.T @ W_O^T + b_O/4, token-major, partial over heads

Constants (iotas, triangular mask, identity) are host-provided inputs so the
GpSimd engine only does DMA descriptor generation and a few broadcasts (its
library swaps between op kinds cost ~6us each).
"""

import sys

for _p in ("/opt/trn_rl_repo",):
    if _p not in sys.path:
        sys.path.insert(0, _p)

import numpy as np

B, S, D, H, DK = 2, 2048, 512, 8, 64
NCORES = 8
HPC = 2            # heads per core
DKH = DK * HPC     # 128: stacked head dim per core
KT = 128           # k-tile (partition tile of keys)
P = 128

_programs = {}


def _consts(s):
    NKT = s // KT
    QCH = min(512, s)
    MW = s // P        # mask columns when loaded partition-major
    RVW = QCH // 32    # columns of the packed rowvalid/denominator layout
    return NKT, QCH, MW, RVW


def make_consts(s):
    """Host-side constant tensors (iotas / masks / identity)."""
    NKT, QCH, MW, RVW = _consts(s)
    p = np.arange(P)
    # cf32: [128, RVW+NKT+1] f32: iota for rv4T compare, iota over k, ones col
    iota_rv = (QCH * (p[:, None] // 32) + (p[:, None] % 32)
               + 32 * np.arange(RVW)[None, :]).astype(np.float32)
    iota_k = (p[:, None] + P * np.arange(NKT)[None, :]).astype(np.float32)
    ones_col = np.ones((P, 1), np.float32)
    cf32 = np.concatenate([iota_rv, iota_k, ones_col], axis=1)
    # cbf: [128, 256] bf16: upper-triangular keep-mask, identity
    import ml_dtypes
    utri = np.triu(np.ones((P, P))).astype(ml_dtypes.bfloat16)
    ident = np.eye(P).astype(ml_dtypes.bfloat16)
    cbf = np.concatenate([utri, ident], axis=1)
    return cf32, cbf


def build_program(s=S):
    import concourse.mybir as mybir
    import concourse.tile as tile
    from concourse import bacc
    from concourse.bass import ts

    dt = mybir.dt
    f32, bf16, i32 = dt.float32, dt.bfloat16, dt.int32
    AF = mybir.ActivationFunctionType
    OP = mybir.AluOpType

    NKT, QCH, MW, RVW = _consts(s)
    NQC = s // QCH              # (must be <= 4 for the denominator packing)
    SCH = min(1024, s)          # q-chunk for score tiles / exp
    PCH = min(512, s)           # q-chunk for projections
    NDC = D // P                # 4 chunks of the contraction dim
    VSTR = 2 * (DK + 1)         # 130: per-k-tile stride in V_aug

    nc = bacc.Bacc("TRN2", target_bir_lowering=False, debug=False,
                   num_devices=NCORES)

    xq_d = nc.dram_tensor("xq", [D, s], f32, kind="ExternalInput")
    xk_d = nc.dram_tensor("xk", [D, s], f32, kind="ExternalInput")
    xv_d = nc.dram_tensor("xv", [D, s], f32, kind="ExternalInput")
    wq_d = nc.dram_tensor("wq", [D, DKH], f32, kind="ExternalInput")
    wk_d = nc.dram_tensor("wk", [D, DKH], f32, kind="ExternalInput")
    wv_d = nc.dram_tensor("wv", [D, DKH], f32, kind="ExternalInput")
    wo_d = nc.dram_tensor("wo", [DKH, D], f32, kind="ExternalInput")
    bqkv_d = nc.dram_tensor("bqkv", [DKH, 3], f32, kind="ExternalInput")
    bo_d = nc.dram_tensor("bo", [D], f32, kind="ExternalInput")
    # masks packed partition-major: [128, 2*MW] i32 (row | col)
    masks_d = nc.dram_tensor("masks", [P, 2 * MW], i32, kind="ExternalInput")
    cf32_d = nc.dram_tensor("cf32", [P, RVW + NKT + 1], f32,
                           kind="ExternalInput")
    cbf_d = nc.dram_tensor("cbf", [P, 2 * P], bf16, kind="ExternalInput")
    out_d = nc.dram_tensor("out", [s, D], f32, kind="ExternalOutput")

    with tile.TileContext(nc) as tc:
        with (
            tc.tile_pool(name="persist", bufs=1) as pe,
            tc.tile_pool(name="pt", bufs=4) as ptp,
            tc.tile_pool(name="sm", bufs=2) as smp,
            tc.tile_pool(name="outp", bufs=3) as outp,
            tc.tile_pool(name="sc", bufs=2, space="PSUM") as scp,
            tc.tile_pool(name="at", bufs=4, space="PSUM") as atp,
        ):
            # ------- small constants via HWDGE (sync) ----------------------
            bqkv = pe.tile([P, 3], f32, tag="bqkv")
            nc.sync.dma_start(bqkv[:], bqkv_d[:])
            masks = pe.tile([P, 2 * MW], i32, tag="masks")
            nc.sync.dma_start(masks[:], masks_d[:])
            cf32 = pe.tile([P, RVW + NKT + 1], f32, tag="cf32")
            nc.sync.dma_start(cf32[:], cf32_d[:])
            cbf = pe.tile([P, 2 * P], bf16, tag="cbf")
            nc.sync.dma_start(cbf[:], cbf_d[:])
            wo_f = pe.tile([P, D], f32, tag="wo_f")
            nc.sync.dma_start(wo_f[:], wo_d[:])
            bo_sb = pe.tile([1, D], f32, tag="bo")
            nc.sync.dma_start(bo_sb[0:1, :], bo_d[None, :])
            iota_rv = cf32[:, 0:RVW]
            iota_k = cf32[:, RVW:RVW + NKT]
            ones_col = cf32[:, RVW + NKT:RVW + NKT + 1]
            utri = cbf[:, 0:P]
            ident = cbf[:, P:2 * P]

            # ------- x^T + weight loads (SWDGE, casts f32->bf16) -----------
            xbf = {}
            wsb = {}
            for nm, xd, wd in (("xq", xq_d, wq_d), ("xk", xk_d, wk_d),
                               ("xv", xv_d, wv_d)):
                w = pe.tile([P, NDC, DKH], bf16, tag="w" + nm[1])
                nc.gpsimd.dma_start(w[:], wd[:].rearrange("(c p) m -> p c m", p=P))
                wsb["w" + nm[1]] = w
                xbf[nm] = []
                for c in range(NDC):
                    t = pe.tile([P, s], bf16, tag=f"{nm}{c}")
                    nc.gpsimd.dma_start(t[:], xd[ts(c, P), :])
                    xbf[nm].append(t)

            # ------- mask lengths + compare masks (wide ops only) ----------
            masks_f = pe.tile([P, 2 * MW], f32, tag="masks_f")
            nc.vector.tensor_copy(out=masks_f[:], in_=masks[:])
            msum = pe.tile([P, 2], f32, tag="msum")
            nc.vector.tensor_reduce(msum[:, 0:1], masks_f[:, 0:MW],
                                    axis=mybir.AxisListType.X, op=OP.add)
            nc.vector.tensor_reduce(msum[:, 1:2], masks_f[:, MW:2 * MW],
                                    axis=mybir.AxisListType.X, op=OP.add)
            # sum over partitions + broadcast back, via tiny f32 matmuls
            lens_ps = scp.tile([P, SCH], f32, tag="sc", name="lens_ps")
            nc.tensor.matmul(lens_ps[0:1, 0:2], ones_col, msum[:, 0:2],
                             start=True, stop=True)
            lens = pe.tile([1, 2], f32, tag="lens")
            nc.vector.tensor_copy(out=lens[0:1, :], in_=lens_ps[0:1, 0:2])
            ones_row = pe.tile([1, P], f32, tag="ones_row")
            nc.vector.memset(ones_row[0:1, :], 1.0)
            lbc_ps = scp.tile([P, SCH], f32, tag="sc", name="lbc_ps")
            nc.tensor.matmul(lbc_ps[:, 0:2], ones_row[0:1, :], lens[0:1, :],
                             start=True, stop=True)
            lens_bc = pe.tile([P, 2], f32, tag="lens_bc")
            nc.vector.tensor_copy(out=lens_bc[:], in_=lbc_ps[:, 0:2])

            # rv4T: rowvalid in the 32x32-transposed packed layout [128, MW]
            rv4T = pe.tile([P, RVW], f32, tag="rv4T")
            nc.vector.tensor_scalar(rv4T[:, :], iota_rv, lens_bc[:, 0:1],
                                    None, OP.is_lt)
            # colmask: 0 valid / -1e30 masked, [128, NKT]
            colmask = pe.tile([P, NKT], f32, tag="colmask")
            nc.vector.tensor_scalar(colmask[:, :], iota_k, lens_bc[:, 1:2],
                                    None, OP.is_lt)
            nc.vector.tensor_scalar(colmask[:, :], colmask[:, :],
                                    1.0, 1e30, OP.subtract, OP.mult)

            den4 = pe.tile([P, QCH], f32, tag="den4")
            nc.vector.memset(den4[:], 1.0)
            den4T = pe.tile([P, QCH], f32, tag="den4T")
            den4T_s = den4T[:].rearrange("p (j b) -> p j b", b=32)[:, :, 0]

            # ---------------- projections ----------------
            QT = pe.tile([P, s], bf16, tag="QT")
            KTt = pe.tile([P, s], bf16, tag="KTt")
            VT = pe.tile([P, s], bf16, tag="VT")
            for dst, wnm, bcol, xnm in (
                (QT, "wq", 0, "xq"),
                (KTt, "wk", 1, "xk"),
                (VT, "wv", 2, "xv"),
            ):
                for j in range(s // PCH):
                    ps = scp.tile([P, SCH], f32, tag="sc", name=f"pj_{wnm}_{j}")
                    for c in range(NDC):
                        nc.tensor.matmul(
                            ps[:, 0:PCH], wsb[wnm][:, c, :],
                            xbf[xnm][c][:, ts(j, PCH)],
                            start=(c == 0), stop=(c == NDC - 1))
                    nc.vector.tensor_scalar(dst[:, ts(j, PCH)], ps[:, 0:PCH],
                                            bqkv[:, bcol:bcol + 1], None,
                                            OP.add)

            # V_aug token-major: per k-tile m, cols [Vh0(64)|1|Vh1(64)|1]
            V_aug = pe.tile([P, NKT * VSTR], bf16, tag="vaug")
            nc.vector.memset(V_aug[:], 1.0)
            for m in range(NKT):
                o = m * VSTR
                tp = scp.tile([P, P], bf16, tag="sc", name=f"tp_{m}")
                nc.tensor.transpose(tp[:, 0:P], VT[:, ts(m, P)], ident)
                nc.vector.tensor_copy(out=V_aug[:, o:o + DK], in_=tp[:, 0:DK])
                nc.vector.tensor_copy(out=V_aug[:, o + DK + 1:o + 2 * DK + 1],
                                      in_=tp[:, DK:2 * DK])

            wo_bf = pe.tile([P, D], bf16, tag="wo_bf")
            nc.vector.tensor_copy(out=wo_bf[:], in_=wo_f[:])
            bo_ps = scp.tile([P, max(SCH, D)], f32, tag="sc", name="bo_ps")
            nc.tensor.matmul(bo_ps[:, 0:D], ones_row[0:1, :], bo_sb[0:1, :],
                             start=True, stop=True)
            bo_bc = pe.tile([P, D], f32, tag="bo_bc")
            nc.vector.tensor_copy(out=bo_bc[:], in_=bo_ps[:, 0:D])

            # ---------------- attention (per head) ----------------
            attnT = pe.tile([P, s], bf16, tag="attnT")

            def emit_out_tile(t):
                ps = scp.tile([P, max(SCH, D)], f32, tag="sc",
                              name=f"out_ps_{t}")
                nc.tensor.matmul(ps[:, 0:D], attnT[:, ts(t, P)], wo_bf[:, :],
                                 start=True, stop=True)
                os_ = outp.tile([P, D], f32, tag="os", name=f"os_{t}")
                nc.vector.tensor_tensor(os_[:, :], ps[:, 0:D], bo_bc[:, :],
                                        OP.add)
                nc.sync.dma_start(out_d[ts(t, P), :], os_[:, :])

            for h in range(HPC):
                hb = h * DK
                aps = [atp.tile([P, QCH], f32, tag="at", name=f"aps_h{h}_{i}")
                       for i in range(NQC)]
                for m in range(NKT):
                    q0 = m * KT
                    width = s - q0
                    PT_m = ptp.tile([P, s], bf16, tag="pt", name=f"pt_{h}_{m}")
                    for off in range(0, width, SCH):
                        n = min(SCH, width - off)
                        ps = scp.tile([P, SCH], f32, tag="sc",
                                      name=f"sc_{h}_{m}_{off}")
                        for o2 in range(0, n, 512):
                            n2 = min(512, n - o2)
                            nc.tensor.matmul(
                                ps[:, o2:o2 + n2], KTt[hb:hb + DK, ts(m, KT)],
                                QT[hb:hb + DK, q0 + off + o2:q0 + off + o2 + n2],
                                start=True, stop=True, skip_group_check=True)
                        nc.scalar.activation(
                            PT_m[:, off:off + n], ps[:, 0:n], AF.Exp,
                            bias=colmask[:, m:m + 1], scale=1.0)
                    # causal mask on the diagonal block
                    nc.vector.tensor_tensor(PT_m[:, 0:P], PT_m[:, 0:P],
                                            utri, OP.mult)
                    # PV accumulate (+ denominator at row 64 via ones column)
                    for qc in range(NQC):
                        aq0 = qc * QCH
                        if q0 >= aq0 + QCH:
                            continue
                        lo = max(aq0, q0)
                        psoff = lo - aq0
                        n = aq0 + QCH - lo
                        m_last = (aq0 + QCH) // KT - 1
                        nc.tensor.matmul(
                            aps[qc][0:DK + 1, psoff:psoff + n],
                            V_aug[:, m * VSTR + h * (DK + 1):
                                  m * VSTR + (h + 1) * (DK + 1)],
                            PT_m[:, lo - q0:lo - q0 + n],
                            start=(m == 0), stop=(m == m_last),
                            skip_group_check=True)
                        if m == m_last:
                            # denominator -> packed row 32*qc
                            nc.vector.tensor_copy(
                                out=den4[32 * qc:32 * qc + 1, :],
                                in_=aps[qc][DK:DK + 1, :])
                # wide reciprocal: transpose, 1/x on the packed lanes, mask
                nc.vector.transpose(den4T[:, :], den4[:, :])
                nc.vector.reciprocal(den4T_s, den4T_s)
                nc.vector.tensor_tensor(den4T_s, den4T_s, rv4T[:, :], OP.mult)
                for qc in range(NQC):
                    rmq = smp.tile([32, QCH], f32, tag="rmq",
                                   name=f"rmq_{h}_{qc}")
                    nc.vector.transpose(rmq[0:32, :],
                                        den4T[32 * qc:32 * qc + 32, :])
                    rbc = smp.tile([DK, QCH], f32, tag="rbc",
                                   name=f"rbc_{h}_{qc}")
                    nc.gpsimd.partition_broadcast(rbc[:, :], rmq[0:1, :])
                    nc.vector.tensor_tensor(
                        attnT[hb:hb + DK, ts(qc, QCH)],
                        aps[qc][0:DK, :], rbc[0:DK, :], OP.mult)
                    if h == HPC - 1:
                        for t in range(qc * QCH // P, (qc + 1) * QCH // P):
                            emit_out_tile(t)

    nc.compile()
    return nc


def _get_program(s=S):
    if s not in _programs:
        _programs[s] = build_program(s)
    return _programs[s]


def pack_masks(row_pad_mask, col_pad_mask, s=S):
    MW = s // P
    r = np.asarray(row_pad_mask, np.int32).reshape(MW, P).T
    c = np.asarray(col_pad_mask, np.int32).reshape(MW, P).T
    return np.ascontiguousarray(np.concatenate([r, c], axis=1))


def make_in_maps(in_Q, in_K, in_V, row_pad_mask, col_pad_mask,
                 W_Q, b_Q, W_K, b_K, W_V, b_V, W_O, b_O):
    """Shard the full inputs into the 8 per-core input maps."""
    f = np.float32
    cf32, cbf = make_consts(S)
    xT = {}
    masks = {}
    for b in range(B):
        xT[b] = tuple(
            np.ascontiguousarray(np.asarray(x[b], dtype=f).T)
            for x in (in_Q, in_K, in_V))
        masks[b] = pack_masks(row_pad_mask[b], col_pad_mask[b])
    per_hp = []
    for hp in range(NCORES // B):
        sl = slice(hp * DKH, (hp + 1) * DKH)
        bqkv = np.stack([np.asarray(b_Q, f)[sl] / 8.0,
                         np.asarray(b_K, f)[sl],
                         np.asarray(b_V, f)[sl]], axis=1)
        per_hp.append(dict(
            wq=np.ascontiguousarray(np.asarray(W_Q, f)[sl, :].T) / 8.0,
            wk=np.ascontiguousarray(np.asarray(W_K, f)[sl, :].T),
            wv=np.ascontiguousarray(np.asarray(W_V, f)[sl, :].T),
            wo=np.ascontiguousarray(np.asarray(W_O, f)[:, sl].T),
            bqkv=np.ascontiguousarray(bqkv),
            bo=np.asarray(b_O, f) / 4.0,
            cf32=cf32, cbf=cbf,
        ))
    in_maps = []
    for c in range(NCORES):
        b, hp = divmod(c, NCORES // B)
        m = dict(per_hp[hp])
        m["xq"], m["xk"], m["xv"] = xT[b]
        m["masks"] = masks[b]
        in_maps.append(m)
    return in_maps


def run(in_maps, trace=False, **trace_kwargs):
    from concourse.bass_utils import run_bass_kernel_spmd
    nc = _get_program()
    return run_bass_kernel_spmd(nc, in_maps, core_ids=list(range(NCORES)),
                                trace=trace, **trace_kwargs)


def kernel(**inputs):
    in_maps = make_in_maps(**inputs)
    res = run(in_maps)
    out = np.zeros((B, S, D), np.float32)
    for c in range(NCORES):
        b = c // (NCORES // B)
        out[b] += res.results[c]["out"]
    return out


# revision 17
# speedup vs baseline: 1.2954x; 1.1798x over previous
"""Trainium2 Bass kernel for nn_MultiHeadAttention (B=2, S=2048, D=512, H=8).

Sharding: 8 cores = 2 batches x 4 head-pairs. Each core computes, for its
batch b and its 2 heads, the full attention and a partial output projection
(row-parallel W_O); the host sums the 4 partial outputs per batch.

Device-side dataflow per core (all matmuls bf16, fp32 accumulation):
  - inputs arrive as x^T [D, S] (feature-major), weights pre-sliced/transposed
  - Q^T, K^T [128, S] feature-major (2 heads stacked on partitions)
  - V token-major via PE transpose, augmented with a ones column per head so
    the PV matmul also produces the softmax denominator for free
  - scores^T [k, q] per k-tile, causal tiles only; exp on ScalarE with the
    column-padding additive mask folded into the activation bias
  - normalization deferred to after PV: the 4 q-chunk denominators are packed
    onto partitions {0,32,64,96}, reciprocal'd in a 32x32-transposed layout
    (avoids 1-lane DVE reciprocals), row-padding mask folded in
  - out = attn^T.T @ W_O^T + b_O/4, token-major, partial over heads

Constants (iotas, triangular mask, identity) are host-provided inputs so the
GpSimd engine only does DMA descriptor generation and a few broadcasts (its
library swaps between op kinds cost ~6us each).
"""

import sys

for _p in ("/opt/trn_rl_repo",):
    if _p not in sys.path:
        sys.path.insert(0, _p)

import numpy as np

B, S, D, H, DK = 2, 2048, 512, 8, 64
NCORES = 8
HPC = 2            # heads per core
DKH = DK * HPC     # 128: stacked head dim per core
KT = 128           # k-tile (partition tile of keys)
P = 128

_programs = {}


def _consts(s):
    NKT = s // KT
    QCH = min(512, s)
    MW = s // P        # mask columns when loaded partition-major
    RVW = QCH // 32    # columns of the packed rowvalid/denominator layout
    return NKT, QCH, MW, RVW


def make_consts(s):
    """Host-side constant tensors (iotas / masks / identity)."""
    NKT, QCH, MW, RVW = _consts(s)
    p = np.arange(P)
    # cf32: [128, RVW+NKT+1] f32: iota for rv4T compare, iota over k, ones col
    iota_rv = (QCH * (p[:, None] // 32) + (p[:, None] % 32)
               + 32 * np.arange(RVW)[None, :]).astype(np.float32)
    iota_k = (p[:, None] + P * np.arange(NKT)[None, :]).astype(np.float32)
    ones_col = np.ones((P, 1), np.float32)
    cf32 = np.concatenate([iota_rv, iota_k, ones_col], axis=1)
    # cbf: [128, 256] bf16: upper-triangular keep-mask, identity
    import ml_dtypes
    utri = np.triu(np.ones((P, P))).astype(ml_dtypes.bfloat16)
    ident = np.eye(P).astype(ml_dtypes.bfloat16)
    cbf = np.concatenate([utri, ident], axis=1)
    return cf32, cbf


def build_program(s=S):
    import concourse.mybir as mybir
    import concourse.tile as tile
    from concourse import bacc
    from concourse.bass import ts

    dt = mybir.dt
    f32, bf16, i32 = dt.float32, dt.bfloat16, dt.int32
    AF = mybir.ActivationFunctionType
    OP = mybir.AluOpType

    NKT, QCH, MW, RVW = _consts(s)
    NQC = s // QCH              # (must be <= 4 for the denominator packing)
    SCH = min(512, s)           # q-chunk for score tiles / exp
    PCH = min(512, s)           # q-chunk for projections
    NDC = D // P                # 4 chunks of the contraction dim
    VSTR = 2 * (DK + 1)         # 130: per-k-tile stride in V_aug

    nc = bacc.Bacc("TRN2", target_bir_lowering=False, debug=False,
                   num_devices=NCORES)

    xq_d = nc.dram_tensor("xq", [D, s], f32, kind="ExternalInput")
    xk_d = nc.dram_tensor("xk", [D, s], f32, kind="ExternalInput")
    xv_d = nc.dram_tensor("xv", [D, s], f32, kind="ExternalInput")
    wq_d = nc.dram_tensor("wq", [D, DKH], f32, kind="ExternalInput")
    wk_d = nc.dram_tensor("wk", [D, DKH], f32, kind="ExternalInput")
    wv_d = nc.dram_tensor("wv", [D, DKH], f32, kind="ExternalInput")
    wo_d = nc.dram_tensor("wo", [DKH, D], f32, kind="ExternalInput")
    bqkv_d = nc.dram_tensor("bqkv", [DKH, 3], f32, kind="ExternalInput")
    bo_d = nc.dram_tensor("bo", [D], f32, kind="ExternalInput")
    # masks packed partition-major: [128, 2*MW] i32 (row | col)
    masks_d = nc.dram_tensor("masks", [P, 2 * MW], i32, kind="ExternalInput")
    cf32_d = nc.dram_tensor("cf32", [P, RVW + NKT + 1], f32,
                           kind="ExternalInput")
    cbf_d = nc.dram_tensor("cbf", [P, 2 * P], bf16, kind="ExternalInput")
    out_d = nc.dram_tensor("out", [s, D], f32, kind="ExternalOutput")

    with tile.TileContext(nc) as tc:
        with (
            tc.tile_pool(name="persist", bufs=1) as pe,
            tc.tile_pool(name="pt", bufs=6) as ptp,
            tc.tile_pool(name="sm", bufs=4) as smp,
            tc.tile_pool(name="outp", bufs=4) as outp,
            tc.tile_pool(name="sc", bufs=4, space="PSUM") as scp,
            tc.tile_pool(name="at", bufs=4, space="PSUM") as atp,
        ):
            # ------- small constants via HWDGE (sync) ----------------------
            bqkv = pe.tile([P, 3], f32, tag="bqkv")
            nc.sync.dma_start(bqkv[:], bqkv_d[:])
            masks = pe.tile([P, 2 * MW], i32, tag="masks")
            nc.sync.dma_start(masks[:], masks_d[:])
            cf32 = pe.tile([P, RVW + NKT + 1], f32, tag="cf32")
            nc.sync.dma_start(cf32[:], cf32_d[:])
            cbf = pe.tile([P, 2 * P], bf16, tag="cbf")
            nc.sync.dma_start(cbf[:], cbf_d[:])
            wo_f = pe.tile([P, D], f32, tag="wo_f")
            nc.sync.dma_start(wo_f[:], wo_d[:])
            bo_sb = pe.tile([1, D], f32, tag="bo")
            nc.sync.dma_start(bo_sb[0:1, :], bo_d[None, :])
            iota_rv = cf32[:, 0:RVW]
            iota_k = cf32[:, RVW:RVW + NKT]
            ones_col = cf32[:, RVW + NKT:RVW + NKT + 1]
            utri = cbf[:, 0:P]
            ident = cbf[:, P:2 * P]

            # ------- x^T + weight loads (SWDGE, casts f32->bf16) -----------
            xbf = {}
            wsb = {}
            for nm, xd, wd in (("xq", xq_d, wq_d), ("xk", xk_d, wk_d),
                               ("xv", xv_d, wv_d)):
                w = pe.tile([P, NDC, DKH], bf16, tag="w" + nm[1])
                nc.gpsimd.dma_start(w[:], wd[:].rearrange("(c p) m -> p c m", p=P))
                wsb["w" + nm[1]] = w
                xbf[nm] = []
                for c in range(NDC):
                    t = pe.tile([P, s], bf16, tag=f"{nm}{c}")
                    nc.gpsimd.dma_start(t[:], xd[ts(c, P), :])
                    xbf[nm].append(t)

            # ------- mask lengths + compare masks (wide ops only) ----------
            masks_f = pe.tile([P, 2 * MW], f32, tag="masks_f")
            nc.vector.tensor_copy(out=masks_f[:], in_=masks[:])
            msum = pe.tile([P, 2], f32, tag="msum")
            nc.vector.tensor_reduce(msum[:, 0:1], masks_f[:, 0:MW],
                                    axis=mybir.AxisListType.X, op=OP.add)
            nc.vector.tensor_reduce(msum[:, 1:2], masks_f[:, MW:2 * MW],
                                    axis=mybir.AxisListType.X, op=OP.add)
            # sum over partitions + broadcast back, via tiny f32 matmuls
            lens_ps = scp.tile([P, SCH], f32, tag="sc", name="lens_ps")
            nc.tensor.matmul(lens_ps[0:1, 0:2], ones_col, msum[:, 0:2],
                             start=True, stop=True)
            lens = pe.tile([1, 2], f32, tag="lens")
            nc.vector.tensor_copy(out=lens[0:1, :], in_=lens_ps[0:1, 0:2])
            ones_row = pe.tile([1, P], f32, tag="ones_row")
            nc.vector.memset(ones_row[0:1, :], 1.0)
            lbc_ps = scp.tile([P, SCH], f32, tag="sc", name="lbc_ps")
            nc.tensor.matmul(lbc_ps[:, 0:2], ones_row[0:1, :], lens[0:1, :],
                             start=True, stop=True)
            lens_bc = pe.tile([P, 2], f32, tag="lens_bc")
            nc.vector.tensor_copy(out=lens_bc[:], in_=lbc_ps[:, 0:2])

            # rv4T: rowvalid in the 32x32-transposed packed layout [128, MW]
            rv4T = pe.tile([P, RVW], f32, tag="rv4T")
            nc.vector.tensor_scalar(rv4T[:, :], iota_rv, lens_bc[:, 0:1],
                                    None, OP.is_lt)
            # colmask: 0 valid / -1e30 masked, [128, NKT]
            colmask = pe.tile([P, NKT], f32, tag="colmask")
            nc.vector.tensor_scalar(colmask[:, :], iota_k, lens_bc[:, 1:2],
                                    None, OP.is_lt)
            nc.vector.tensor_scalar(colmask[:, :], colmask[:, :],
                                    1.0, 1e30, OP.subtract, OP.mult)

            den4 = pe.tile([P, QCH], f32, tag="den4")
            nc.vector.memset(den4[:], 1.0)
            den4T = pe.tile([P, QCH], f32, tag="den4T")
            den4T_s = den4T[:].rearrange("p (j b) -> p j b", b=32)[:, :, 0]

            # ---------------- projections ----------------
            QT = pe.tile([P, s], bf16, tag="QT")
            KTt = pe.tile([P, s], bf16, tag="KTt")
            VT = pe.tile([P, s], bf16, tag="VT")
            for dst, wnm, bcol, xnm in (
                (QT, "wq", 0, "xq"),
                (KTt, "wk", 1, "xk"),
                (VT, "wv", 2, "xv"),
            ):
                for j in range(s // PCH):
                    ps = scp.tile([P, SCH], f32, tag="sc", name=f"pj_{wnm}_{j}")
                    for c in range(NDC):
                        nc.tensor.matmul(
                            ps[:, 0:PCH], wsb[wnm][:, c, :],
                            xbf[xnm][c][:, ts(j, PCH)],
                            start=(c == 0), stop=(c == NDC - 1))
                    nc.vector.tensor_scalar(dst[:, ts(j, PCH)], ps[:, 0:PCH],
                                            bqkv[:, bcol:bcol + 1], None,
                                            OP.add)

            # V_aug token-major: per k-tile m, cols [Vh0(64)|1|Vh1(64)|1]
            V_aug = pe.tile([P, NKT * VSTR], bf16, tag="vaug")
            nc.vector.memset(V_aug[:], 1.0)
            for m in range(NKT):
                o = m * VSTR
                tp = scp.tile([P, P], bf16, tag="sc", name=f"tp_{m}")
                nc.tensor.transpose(tp[:, 0:P], VT[:, ts(m, P)], ident)
                nc.vector.tensor_copy(out=V_aug[:, o:o + DK], in_=tp[:, 0:DK])
                nc.vector.tensor_copy(out=V_aug[:, o + DK + 1:o + 2 * DK + 1],
                                      in_=tp[:, DK:2 * DK])

            wo_bf = pe.tile([P, D], bf16, tag="wo_bf")
            nc.vector.tensor_copy(out=wo_bf[:], in_=wo_f[:])
            bo_ps = scp.tile([P, max(SCH, D)], f32, tag="sc", name="bo_ps")
            nc.tensor.matmul(bo_ps[:, 0:D], ones_row[0:1, :], bo_sb[0:1, :],
                             start=True, stop=True)
            bo_bc = pe.tile([P, D], f32, tag="bo_bc")
            nc.vector.tensor_copy(out=bo_bc[:], in_=bo_ps[:, 0:D])

            # ---------------- attention (per head) ----------------
            attnT = pe.tile([P, s], bf16, tag="attnT")

            def emit_out_tile(t):
                ps = scp.tile([P, max(SCH, D)], f32, tag="sc",
                              name=f"out_ps_{t}")
                nc.tensor.matmul(ps[:, 0:D], attnT[:, ts(t, P)], wo_bf[:, :],
                                 start=True, stop=True)
                os_ = outp.tile([P, D], f32, tag="os", name=f"os_{t}")
                nc.vector.tensor_tensor(os_[:, :], ps[:, 0:D], bo_bc[:, :],
                                        OP.add)
                nc.sync.dma_start(out_d[ts(t, P), :], os_[:, :])

            for h in range(HPC):
                hb = h * DK
                aps = [atp.tile([P, QCH], f32, tag="at", name=f"aps_h{h}_{i}")
                       for i in range(NQC)]
                for m in range(NKT):
                    q0 = m * KT
                    width = s - q0
                    PT_m = ptp.tile([P, s], bf16, tag="pt", name=f"pt_{h}_{m}")
                    for off in range(0, width, SCH):
                        n = min(SCH, width - off)
                        ps = scp.tile([P, SCH], f32, tag="sc",
                                      name=f"sc_{h}_{m}_{off}")
                        for o2 in range(0, n, 512):
                            n2 = min(512, n - o2)
                            nc.tensor.matmul(
                                ps[:, o2:o2 + n2], KTt[hb:hb + DK, ts(m, KT)],
                                QT[hb:hb + DK, q0 + off + o2:q0 + off + o2 + n2],
                                start=True, stop=True, skip_group_check=True)
                        nc.scalar.activation(
                            PT_m[:, off:off + n], ps[:, 0:n], AF.Exp,
                            bias=colmask[:, m:m + 1], scale=1.0)
                    # causal mask on the diagonal block
                    nc.vector.tensor_tensor(PT_m[:, 0:P], PT_m[:, 0:P],
                                            utri, OP.mult)
                    # PV accumulate (+ denominator at row 64 via ones column)
                    for qc in range(NQC):
                        aq0 = qc * QCH
                        if q0 >= aq0 + QCH:
                            continue
                        lo = max(aq0, q0)
                        psoff = lo - aq0
                        n = aq0 + QCH - lo
                        m_last = (aq0 + QCH) // KT - 1
                        nc.tensor.matmul(
                            aps[qc][0:DK + 1, psoff:psoff + n],
                            V_aug[:, m * VSTR + h * (DK + 1):
                                  m * VSTR + (h + 1) * (DK + 1)],
                            PT_m[:, lo - q0:lo - q0 + n],
                            start=(m == 0), stop=(m == m_last),
                            skip_group_check=True)
                        if m == m_last:
                            # denominator -> packed row 32*qc
                            nc.vector.tensor_copy(
                                out=den4[32 * qc:32 * qc + 1, :],
                                in_=aps[qc][DK:DK + 1, :])
                # wide reciprocal: transpose, 1/x on the packed lanes, mask
                nc.vector.transpose(den4T[:, :], den4[:, :])
                nc.vector.reciprocal(den4T_s, den4T_s)
                nc.vector.tensor_tensor(den4T_s, den4T_s, rv4T[:, :], OP.mult)
                for qc in range(NQC):
                    rmq = smp.tile([32, QCH], f32, tag="rmq",
                                   name=f"rmq_{h}_{qc}")
                    nc.vector.transpose(rmq[0:32, :],
                                        den4T[32 * qc:32 * qc + 32, :])
                    rbc = smp.tile([DK, QCH], f32, tag="rbc",
                                   name=f"rbc_{h}_{qc}")
                    nc.gpsimd.partition_broadcast(rbc[:, :], rmq[0:1, :])
                    nc.vector.tensor_tensor(
                        attnT[hb:hb + DK, ts(qc, QCH)],
                        aps[qc][0:DK, :], rbc[0:DK, :], OP.mult)
                    if h == HPC - 1:
                        for t in range(qc * QCH // P, (qc + 1) * QCH // P):
                            emit_out_tile(t)

    nc.compile()
    return nc


def _get_program(s=S):
    if s not in _programs:
        _programs[s] = build_program(s)
    return _programs[s]


def pack_masks(row_pad_mask, col_pad_mask, s=S):
    MW = s // P
    r = np.asarray(row_pad_mask, np.int32).reshape(MW, P).T
    c = np.asarray(col_pad_mask, np.int32).reshape(MW, P).T
    return np.ascontiguousarray(np.concatenate([r, c], axis=1))


def make_in_maps(in_Q, in_K, in_V, row_pad_mask, col_pad_mask,
                 W_Q, b_Q, W_K, b_K, W_V, b_V, W_O, b_O):
    """Shard the full inputs into the 8 per-core input maps."""
    f = np.float32
    cf32, cbf = make_consts(S)
    xT = {}
    masks = {}
    for b in range(B):
        xT[b] = tuple(
            np.ascontiguousarray(np.asarray(x[b], dtype=f).T)
            for x in (in_Q, in_K, in_V))
        masks[b] = pack_masks(row_pad_mask[b], col_pad_mask[b])
    per_hp = []
    for hp in range(NCORES // B):
        sl = slice(hp * DKH, (hp + 1) * DKH)
        bqkv = np.stack([np.asarray(b_Q, f)[sl] / 8.0,
                         np.asarray(b_K, f)[sl],
                         np.asarray(b_V, f)[sl]], axis=1)
        per_hp.append(dict(
            wq=np.ascontiguousarray(np.asarray(W_Q, f)[sl, :].T) / 8.0,
            wk=np.ascontiguousarray(np.asarray(W_K, f)[sl, :].T),
            wv=np.ascontiguousarray(np.asarray(W_V, f)[sl, :].T),
            wo=np.ascontiguousarray(np.asarray(W_O, f)[:, sl].T),
            bqkv=np.ascontiguousarray(bqkv),
            bo=np.asarray(b_O, f) / 4.0,
            cf32=cf32, cbf=cbf,
        ))
    in_maps = []
    for c in range(NCORES):
        b, hp = divmod(c, NCORES // B)
        m = dict(per_hp[hp])
        m["xq"], m["xk"], m["xv"] = xT[b]
        m["masks"] = masks[b]
        in_maps.append(m)
    return in_maps


def run(in_maps, trace=False, **trace_kwargs):
    from concourse.bass_utils import run_bass_kernel_spmd
    nc = _get_program()
    return run_bass_kernel_spmd(nc, in_maps, core_ids=list(range(NCORES)),
                                trace=trace, **trace_kwargs)


def kernel(**inputs):
    in_maps = make_in_maps(**inputs)
    res = run(in_maps)
    out = np.zeros((B, S, D), np.float32)
    for c in range(NCORES):
        b = c // (NCORES // B)
        out[b] += res.results[c]["out"]
    return out
